# revision 8
# baseline (speedup 1.0000x reference)
"""Trainium2 Bass kernel for one Tacotron2-style decoder iteration.

Strategy: data-parallel over batch (B=256 -> 32 per core x 8 cores), all
weights replicated. Device math uses bf16 matmul operands with fp32 PSUM
accumulation; states/softmax stay fp32. Activations/weights are laid out
host-side in feature-major ("transposed") order so every engine op runs with
features on the SBUF partition dim.

Host-side work is limited to layout prep on constants/inputs: transposes,
dtype casts, concatenation (bias rows appended as extra matmul K rows),
conv->matmul folding of the location-attention weights, and im2col of the
attention-weight stack (pure gather).
"""

import numpy as np

import concourse.bass as bass
import concourse.mybir as mybir
from concourse import masks
from concourse.bass_utils import run_bass_kernel_spmd
from concourse.tile import TileContext

# ---------------------------------------------------------------- constants
B, T = 256, 512
MEL, PRE, ENC, ARNN, DRNN, ATT, LF, LK = 80, 256, 512, 1024, 1024, 128, 32, 31
PAD = (LK - 1) // 2
NCORES = 8
BL = B // NCORES  # 32 batch rows per core

K1 = 15   # LSTM1 contraction tiles: (256+512+1024+2 -> pad 1920)/128
K2 = 21   # LSTM2: (1024+512+1024+2 -> pad 2688)/128
KP = 13   # proj:  (1024+512+1 -> pad 1664)/128
NM = 32   # gate M-tiles (4096/128)
KH = ARNN // 128  # 8 hidden-state k-tiles

F32 = mybir.dt.float32
BF16 = mybir.dt.bfloat16

def _prenet_masks():
    """The reference regenerates its Bernoulli(0.5) prenet masks from
    jax.random.key(42) at call time; the realized bits depend on the jax
    backend, so compute them the same way (default backend) at run time."""
    import jax
    key = jax.random.key(42)
    return [
        np.asarray(
            jax.random.uniform(jax.random.fold_in(key, li), (PRE,)) <= 0.5
        ).astype(np.float32)
        for li in range(2)
    ]


def _bf(x):
    import ml_dtypes
    return np.ascontiguousarray(x).astype(ml_dtypes.bfloat16)


def _ktiles_pm(x, nk):
    """[nk*128, C] -> [128, nk, C] (partition-major k-tile layout)."""
    c = x.shape[1]
    return np.ascontiguousarray(x.reshape(nk, 128, c).transpose(1, 0, 2))


# ---------------------------------------------------------------- host prep
def _prep_shared(inp):
    """Layout prep shared across cores (weights). Returns dict of arrays."""
    pm0, pm1 = _prenet_masks()
    m0 = pm0 * 2.0
    m1 = pm1 * 2.0
    w1pT = _bf((inp["prenet_w1"] * m0[:, None]).T)            # [80, 256]
    w2pT = _ktiles_pm(_bf((inp["prenet_w2"] * m1[:, None]).T), 2)  # [128,2,256]

    def lstm_tiles(w_ih, w_hh, b_ih, b_hh, nk):
        W = np.concatenate(
            [w_ih.T, w_hh.T, b_ih[None], b_hh[None],
             np.zeros((nk * 128 - w_ih.shape[1] - w_hh.shape[1] - 2,
                       w_ih.shape[0]), np.float32)], 0)        # [nk*128, 4096]
        W = _bf(W)
        # [32 m][128 p][nk][128 mcol] so each m-tile is one contiguous DMA
        out = np.empty((NM, 128, nk, 128), W.dtype)
        for m in range(NM):
            blk = W[:, m * 128:(m + 1) * 128]                  # [nk*128, 128]
            out[m] = blk.reshape(nk, 128, 128).transpose(1, 0, 2)
        return out

    W1t = lstm_tiles(inp["arnn_w_ih"], inp["arnn_w_hh"],
                     inp["arnn_b_ih"], inp["arnn_b_hh"], K1)
    W2t = lstm_tiles(inp["drnn_w_ih"], inp["drnn_w_hh"],
                     inp["drnn_b_ih"], inp["drnn_b_hh"], K2)

    qwT = _ktiles_pm(_bf(inp["q_w"].T), KH)                    # [128, 8, 128]
    M2 = inp["loc_lin_w"] @ inp["loc_conv_w"].reshape(LF, 2 * LK)  # [128, 62]
    M2T = _bf(M2.T)                                            # [62, 128]
    v_col = _bf(inp["v_w"][0].reshape(ATT, 1))                 # [128, 1]

    PJ = np.concatenate(
        [np.concatenate([inp["proj_w"], inp["gate_w"]], 0).T,
         np.concatenate([inp["proj_b"], inp["gate_b"]])[None],
         np.zeros((KP * 128 - 1537, 81), np.float32)], 0)      # [1664, 81]
    pjT = _ktiles_pm(_bf(PJ), KP)                              # [128, 13, 81]

    return dict(w1pT=w1pT, w2pT=w2pT, W1t=W1t, W2t=W2t, qwT=qwT,
                M2T=M2T, v_col=v_col, pjT=pjT)


def _prep_percore(inp):
    """Per-core activation shards. Returns list of dicts (one per core)."""
    # im2col X[(c,k), b, t] = awc[b, c, t+k-15], zero padded
    awc = np.stack([inp["attention_weights"], inp["attention_weights_cum"]], 1)
    X = np.zeros((2 * LK, B, T), np.float32)
    for c in range(2):
        for k in range(LK):
            lo = max(0, PAD - k)
            hi = min(T, T + PAD - k)
            X[c * LK + k, :, lo:hi] = awc[:, c, lo + k - PAD: hi + k - PAD]
    Xb = _bf(X)

    pmT = _bf(inp["processed_memory"].transpose(2, 0, 1))      # [128, B, T]
    memb = _bf(inp["memory"])                                  # [B, T, ENC]

    dinT = np.ascontiguousarray(inp["decoder_input"].T)        # [80, B]
    actxT = _ktiles_pm(np.ascontiguousarray(inp["attention_context"].T), 4)
    ahT = _ktiles_pm(np.ascontiguousarray(inp["attention_hidden"].T), KH)
    acT = _ktiles_pm(np.ascontiguousarray(inp["attention_cell"].T), KH)
    dhT = _ktiles_pm(np.ascontiguousarray(inp["decoder_hidden"].T), KH)
    dcT = _ktiles_pm(np.ascontiguousarray(inp["decoder_cell"].T), KH)

    shards = []
    for c in range(NCORES):
        s = slice(c * BL, (c + 1) * BL)
        shards.append(dict(
            X=np.ascontiguousarray(Xb[:, s, :]).reshape(2 * LK, BL * T),
            pmT=np.ascontiguousarray(pmT[:, s, :]).reshape(128, BL * T),
            mem=np.ascontiguousarray(memb[s]),
            dinT=np.ascontiguousarray(dinT[:, s]),
            actxT=np.ascontiguousarray(actxT[:, :, s]),
            ahT_in=np.ascontiguousarray(ahT[:, :, s]),
            acT_in=np.ascontiguousarray(acT[:, :, s]),
            dhT_in=np.ascontiguousarray(dhT[:, :, s]),
            dcT_in=np.ascontiguousarray(dcT[:, :, s]),
            awcum=np.ascontiguousarray(inp["attention_weights_cum"][s]),
        ))
    return shards


# ---------------------------------------------------------------- bass build
def build_kernel():
    nc = bass.Bass()
    p = {}
    def din(name, shape, dtype):
        p[name] = nc.declare_dram_parameter(name, list(shape), dtype, isOutput=False)
        return p[name]
    def dout(name, shape, dtype):
        p[name] = nc.declare_dram_parameter(name, list(shape), dtype, isOutput=True)
        return p[name]

    # inputs
    din("W1t", (NM, 128, K1, 128), BF16)
    din("W2t", (NM, 128, K2, 128), BF16)
    din("w1pT", (80, 256), BF16)
    din("w2pT", (128, 2, 256), BF16)
    din("qwT", (128, KH, 128), BF16)
    din("M2T", (2 * LK, 128), BF16)
    din("v_col", (ATT, 1), BF16)
    din("pjT", (128, KP, 81), BF16)
    din("X", (2 * LK, BL * T), BF16)
    din("pmT", (128, BL * T), BF16)
    din("mem", (BL, T, ENC), BF16)
    din("dinT", (80, BL), F32)
    din("actxT", (128, 4, BL), F32)
    din("ahT_in", (128, KH, BL), F32)
    din("acT_in", (128, KH, BL), F32)
    din("dhT_in", (128, KH, BL), F32)
    din("dcT_in", (128, KH, BL), F32)
    din("awcum", (BL, T), F32)
    # outputs
    dout("o_proj", (81, BL), F32)
    dout("o_ahT", (128, KH, BL), F32)
    dout("o_acT", (128, KH, BL), F32)
    dout("o_dhT", (128, KH, BL), F32)
    dout("o_dcT", (128, KH, BL), F32)
    dout("o_aw", (BL, T), F32)
    dout("o_awcum", (BL, T), F32)
    dout("o_ctx", (BL, ENC), F32)

    AF = mybir.ActivationFunctionType

    with TileContext(nc) as tc:
        with (
            tc.tile_pool(name="persist", bufs=1) as pp,
            tc.tile_pool(name="wpool", bufs=3) as wp,
            tc.tile_pool(name="big", bufs=1) as bigp,
            tc.tile_pool(name="stream", bufs=8) as sp,
            tc.tile_pool(name="th", bufs=4) as thp,
            tc.tile_pool(name="psA", bufs=1, space="PSUM") as psA,
            tc.tile_pool(name="psB", bufs=2, space="PSUM") as psB,
        ):
            # ---- persistent small tiles
            ident_bf = pp.tile([128, 128], BF16)
            masks.make_identity(nc, ident_bf[:])
            ident_f32 = pp.tile([32, 32], F32)
            masks.make_identity(nc, ident_f32[:])

            rhs1 = pp.tile([128, K1, BL], BF16)
            rhs2 = pp.tile([128, K2, BL], BF16)
            rhs_hc = pp.tile([128, KP, BL], BF16)
            # bias-ones rows: rhs1[k=14] rows 0-1; rhs2[k=20] rows 0-1;
            # rhs_hc[k=12] row 0. Zero the rest of those chunks.
            nc.gpsimd.memset(rhs1[:, K1 - 1, :], 0.0)
            nc.gpsimd.memset(rhs2[:, K2 - 1, :], 0.0)
            nc.gpsimd.memset(rhs_hc[:, KP - 1, :], 0.0)
            nc.gpsimd.memset(rhs1[0:2, K1 - 1, :], 1.0)
            nc.gpsimd.memset(rhs2[0:2, K2 - 1, :], 1.0)
            nc.gpsimd.memset(rhs_hc[0:1, KP - 1, :], 1.0)

            # ---- load small inputs
            w1p_sb = pp.tile([80, 256], BF16)
            nc.sync.dma_start(out=w1p_sb[:], in_=p["w1pT"][:])
            w2p_sb = pp.tile([128, 2, 256], BF16)
            nc.sync.dma_start(out=w2p_sb[:], in_=p["w2pT"][:])
            qw_sb = pp.tile([128, KH, 128], BF16)
            nc.sync.dma_start(out=qw_sb[:], in_=p["qwT"][:])
            m2_sb = pp.tile([2 * LK, 128], BF16)
            nc.sync.dma_start(out=m2_sb[:], in_=p["M2T"][:])
            v_sb = pp.tile([ATT, 1], BF16)
            nc.sync.dma_start(out=v_sb[:], in_=p["v_col"][:])
            pj_sb = pp.tile([128, KP, 81], BF16)
            nc.sync.dma_start(out=pj_sb[:], in_=p["pjT"][:])

            din_sb = pp.tile([80, BL], F32)
            nc.sync.dma_start(out=din_sb[:], in_=p["dinT"][:])
            actx_sb = pp.tile([128, 4, BL], F32)
            nc.sync.dma_start(out=actx_sb[:], in_=p["actxT"][:])
            ah_in_sb = pp.tile([128, KH, BL], F32)
            nc.sync.dma_start(out=ah_in_sb[:], in_=p["ahT_in"][:])
            ac_in_sb = pp.tile([128, KH, BL], F32)
            nc.sync.dma_start(out=ac_in_sb[:], in_=p["acT_in"][:])
            dh_in_sb = pp.tile([128, KH, BL], F32)
            nc.sync.dma_start(out=dh_in_sb[:], in_=p["dhT_in"][:])
            dc_in_sb = pp.tile([128, KH, BL], F32)
            nc.sync.dma_start(out=dc_in_sb[:], in_=p["dcT_in"][:])
            awcum_sb = pp.tile([BL, T], F32)
            nc.sync.dma_start(out=awcum_sb[:], in_=p["awcum"][:])

            # big attention operands (prefetched during LSTM1)
            x_sb = bigp.tile([2 * LK, BL * T], BF16)
            nc.sync.dma_start(out=x_sb[:], in_=p["X"][:])
            pm_sb = bigp.tile([128, BL * T], BF16)
            nc.sync.dma_start(out=pm_sb[:], in_=p["pmT"][:])

            # ---- prenet -> rhs1 chunks 0..1
            din_bf = pp.tile([80, BL], BF16)
            nc.vector.tensor_copy(din_bf[:], din_sb[:])
            ps_x = psB.tile([128, 2, BL], F32, tag="sm")
            for m in range(2):
                nc.tensor.matmul(ps_x[:, m, :], w1p_sb[:, m * 128:(m + 1) * 128],
                                 din_bf[:], start=True, stop=True)
            x1_bf = pp.tile([128, 2, BL], BF16)
            nc.scalar.activation(x1_bf[:], ps_x[:], AF.Relu)
            ps_x2 = psB.tile([128, 2, BL], F32, tag="sm")
            for m in range(2):
                for k in range(2):
                    nc.tensor.matmul(ps_x2[:, m, :],
                                     w2p_sb[:, k, m * 128:(m + 1) * 128],
                                     x1_bf[:, k, :], start=(k == 0), stop=(k == 1))
            nc.scalar.activation(rhs1[:, 0:2, :], ps_x2[:], AF.Relu)

            # rhs1 chunks 2..5 = attention_context^T, 6..13 = ah_in^T (bf16)
            nc.vector.tensor_copy(rhs1[:, 2:6, :], actx_sb[:])
            nc.vector.tensor_copy(rhs1[:, 6:6 + KH, :], ah_in_sb[:])

            # ---- LSTM cell helper
            def lstm(Wdram, nk, rhs, c_in_sb, o_cT, o_hT, h_bf_dst):
                ps_g = psA.tile([128, NM * BL], F32, tag="gates")  # [128, 1024] = 2 banks
                for m in range(NM):
                    w_sb = wp.tile([128, nk * 128], BF16, tag="wtile")
                    nc.sync.dma_start(out=w_sb[:], in_=Wdram[m])
                    wv = w_sb[:].rearrange("p (k c) -> p k c", k=nk)
                    for k in range(nk):
                        nc.tensor.matmul(ps_g[:, m * BL:(m + 1) * BL],
                                         wv[:, k, :], rhs[:, k, :],
                                         start=(k == 0), stop=(k == nk - 1))
                H = KH * BL  # 256 cols per gate group
                sig_if = pp.tile([128, 2 * H], F32, tag=f"sif{nk}")
                nc.scalar.activation(sig_if[:], ps_g[:, 0:2 * H], AF.Sigmoid)
                tan_g = pp.tile([128, H], F32, tag=f"tg{nk}")
                nc.scalar.activation(tan_g[:], ps_g[:, 2 * H:3 * H], AF.Tanh)
                sig_o = pp.tile([128, H], F32, tag=f"so{nk}")
                nc.scalar.activation(sig_o[:], ps_g[:, 3 * H:4 * H], AF.Sigmoid)
                c_new = pp.tile([128, KH, BL], F32, tag=f"cn{nk}")
                ig = pp.tile([128, H], F32, tag=f"ig{nk}")
                nc.vector.tensor_mul(ig[:], sig_if[:, 0:H], tan_g[:])
                cin_flat = c_in_sb[:].rearrange("p k c -> p (k c)")
                fc = pp.tile([128, H], F32, tag=f"fc{nk}")
                nc.vector.tensor_mul(fc[:], sig_if[:, H:2 * H], cin_flat)
                cflat = c_new[:].rearrange("p k c -> p (k c)")
                nc.vector.tensor_add(cflat, ig[:], fc[:])
                nc.sync.dma_start(out=o_cT[:], in_=c_new[:])
                tc2 = pp.tile([128, H], F32, tag=f"tc{nk}")
                nc.scalar.activation(tc2[:], cflat, AF.Tanh)
                h_new = pp.tile([128, KH, BL], F32, tag=f"hn{nk}")
                hflat = h_new[:].rearrange("p k c -> p (k c)")
                nc.vector.tensor_mul(hflat, sig_o[:], tc2[:])
                nc.sync.dma_start(out=o_hT[:], in_=h_new[:])
                nc.vector.tensor_copy(h_bf_dst, h_new[:])  # cast to bf16
                return h_new

            lstm(p["W1t"], K1, rhs1, ac_in_sb, p["o_acT"], p["o_ahT"],
                 rhs2[:, 0:KH, :])

            # ---- attention
            # pq = q_w @ ah  -> [128, BL] fp32 (per-partition bias for tanh)
            ps_pq = psB.tile([128, BL], F32, tag="sm")
            for k in range(KH):
                nc.tensor.matmul(ps_pq[:], qw_sb[:, k, :], rhs2[:, k, :],
                                 start=(k == 0), stop=(k == KH - 1))
            pq_sb = pp.tile([128, BL], F32)
            nc.scalar.activation(pq_sb[:], ps_pq[:], AF.Copy)

            # v_diag[:, j, j] = v ; zero elsewhere. The energies matmul for
            # batch b then uses column block b so e-rows land on partition b
            # of one shared PSUM bank (softmax layout), accumulating zeros
            # into the other 31 rows.
            v_diag = pp.tile([ATT, BL * BL], BF16)
            nc.gpsimd.memset(v_diag[:], 0.0)
            nc.vector.tensor_copy(v_diag[:][:, ::BL + 1],
                                  v_sb[:].broadcast_to((ATT, BL)))
            dv = v_diag[:].rearrange("p (b j) -> p b j", b=BL)

            # energies: e[b, t] accumulated into one [BL, T] PSUM bank
            ps_en = psA.tile([BL, T], F32, tag="enat")
            xv = x_sb[:].rearrange("p (b t) -> p b t", b=BL)
            pmv = pm_sb[:].rearrange("p (b t) -> p b t", b=BL)
            for b in range(BL):
                ps_arg = psB.tile([128, T], F32, tag="earg")
                nc.tensor.matmul(ps_arg[:], m2_sb[:], xv[:, b, :],
                                 start=True, stop=False)
                nc.tensor.matmul(ps_arg[:], ident_bf[:], pmv[:, b, :],
                                 start=False, stop=True)
                th = thp.tile([128, T], BF16, tag="th")
                nc.scalar.activation(th[:], ps_arg[:], AF.Tanh,
                                     bias=pq_sb[:, b:b + 1])
                nc.tensor.matmul(ps_en[:], dv[:, b, :], th[:],
                                 start=(b == 0), stop=(b == BL - 1),
                                 skip_group_check=True)

            # softmax over t (free dim), rows = batch
            mx = pp.tile([BL, 1], F32)
            nc.vector.reduce_max(mx[:], ps_en[:], axis=mybir.AxisListType.X,
                                 negate=True)
            aw_sb = pp.tile([BL, T], F32)
            sm = pp.tile([BL, 1], F32)
            nc.scalar.activation(aw_sb[:], ps_en[:], AF.Exp, bias=mx[:],
                                 accum_out=sm[:])
            inv = pp.tile([BL, 1], F32)
            nc.vector.reciprocal(inv[:], sm[:])
            nc.vector.tensor_scalar_mul(aw_sb[:], aw_sb[:], inv[:])
            nc.sync.dma_start(out=p["o_aw"][:], in_=aw_sb[:])
            awcum_o = pp.tile([BL, T], F32)
            nc.vector.tensor_add(awcum_o[:], awcum_sb[:], aw_sb[:])
            nc.sync.dma_start(out=p["o_awcum"][:], in_=awcum_o[:])

            # aw^T [512, 32] as [128, 4, 32] bf16 via PE transpose
            awT_bf = pp.tile([128, 4, BL], BF16)
            for c in range(4):
                ps_t = psB.tile([128, BL], F32, tag="sm")
                nc.tensor.transpose(ps_t[:], aw_sb[:, c * 128:(c + 1) * 128],
                                    ident_f32[:])
                nc.vector.tensor_copy(awT_bf[:, c, :], ps_t[:])

            # aw^T diag blocks: awd[:, kt, b, j] = aw^T[tchunk kt, b] iff j==b.
            # ctx rows then accumulate straight into a [BL, ENC] PSUM bank.
            awd = pp.tile([128, 4 * BL * BL], BF16)
            nc.gpsimd.memset(awd[:], 0.0)
            awd_diag = awd[:].rearrange("p (kt r) -> p kt r", kt=4)[:, :, ::BL + 1]
            nc.vector.tensor_copy(awd_diag, awT_bf[:])
            awdv = awd[:].rearrange("p (kt b j) -> p kt b j", kt=4, b=BL)

            ps_ctx = psA.tile([BL, ENC], F32, tag="ctx")
            for b in range(BL):
                mem_b = sp.tile([128, 4, ENC], BF16, tag="memb")
                mv = p["mem"][b].rearrange("(k p) e -> k p e", p=128)
                for kt in range(4):
                    nc.sync.dma_start(out=mem_b[:, kt, :], in_=mv[kt])
                    nc.tensor.matmul(ps_ctx[:], awdv[:, kt, b, :],
                                     mem_b[:, kt, :],
                                     start=(b == 0 and kt == 0),
                                     stop=(b == BL - 1 and kt == 3),
                                     skip_group_check=True)
            ctx_sb = pp.tile([BL, ENC], F32)
            nc.scalar.activation(ctx_sb[:], ps_ctx[:], AF.Copy)
            nc.sync.dma_start(out=p["o_ctx"][:], in_=ctx_sb[:])

            # ctx^T -> rhs2 chunks 8..11 (bf16)
            for c in range(4):
                ps_t = psB.tile([128, BL], F32, tag="sm")
                nc.tensor.transpose(ps_t[:], ctx_sb[:, c * 128:(c + 1) * 128],
                                    ident_f32[:])
                nc.vector.tensor_copy(rhs2[:, KH + c, :], ps_t[:])
            # rhs2 chunks 12..19 = dh_in^T
            nc.vector.tensor_copy(rhs2[:, KH + 4:KH + 4 + KH, :], dh_in_sb[:])

            # ---- LSTM2
            dh_new = lstm(p["W2t"], K2, rhs2, dc_in_sb, p["o_dcT"], p["o_dhT"],
                          rhs_hc[:, 0:KH, :])
            # rhs_hc chunks 8..11 = ctx^T
            nc.vector.tensor_copy(rhs_hc[:, KH:KH + 4, :], rhs2[:, KH:KH + 4, :])

            # ---- projection + gate
            ps_pr = psB.tile([81, BL], F32, tag="sm")
            for k in range(KP):
                nc.tensor.matmul(ps_pr[:], pj_sb[:, k, 0:81], rhs_hc[:, k, :],
                                 start=(k == 0), stop=(k == KP - 1))
            pr_sb = pp.tile([81, BL], F32)
            nc.scalar.activation(pr_sb[:], ps_pr[:], AF.Copy)
            nc.sync.dma_start(out=p["o_proj"][:], in_=pr_sb[:])

    _fix_excess_waits(nc)
    return nc


def _fix_excess_waits(nc):
    """This walrus build rejects >1 sync-wait on an instruction; spill extras
    into standalone InstEventSemaphore insts placed just before (same engine,
    so the stall semantics are identical)."""
    for f in nc.m.functions:
        for bb in f.blocks:
            insts = list(bb.instructions)
            out = []
            changed = False
            for ins in insts:
                si = ins.sync_info
                if (si is not None and si.on_wait and len(si.on_wait) > 1
                        and type(ins).__name__ != "InstEventSemaphore"):
                    waits = list(si.on_wait)
                    for j, w in enumerate(waits[1:]):
                        wi = mybir.InstEventSemaphore(
                            name=f"{ins.name}-xw{j}", ins=[], outs=[])
                        wi.engine = ins.engine
                        wi.sync_info = mybir.SyncInfo(on_wait=[w], on_update=[])
                        nc.register_instruction(wi, overwrite=True)
                        out.append(wi)
                        changed = True
                    ins.sync_info = mybir.SyncInfo(
                        on_wait=waits[:1], on_update=list(si.on_update or []))
                out.append(ins)
            if changed:
                bb.instructions = out


_CACHED = {}


def kernel(**inputs):
    inputs = {k: np.asarray(v) for k, v in inputs.items()}
    if "nc" not in _CACHED:
        _CACHED["nc"] = build_kernel()
    nc = _CACHED["nc"]

    shared = _prep_shared(inputs)
    shards = _prep_percore(inputs)
    in_maps = [dict(shared, **sh) for sh in shards]
    res = run_bass_kernel_spmd(nc, in_maps, list(range(NCORES)))

    def _unT(chunks):  # [128, KH, BL] per core -> [B, 1024]
        return np.concatenate(
            [r.transpose(1, 0, 2).reshape(ARNN, BL).T for r in chunks], 0)

    rs = res.results
    dec_out = np.concatenate([r["o_proj"][:80].T for r in rs], 0)
    gate = np.concatenate([r["o_proj"][80:81].T for r in rs], 0)
    ah = _unT([r["o_ahT"] for r in rs])
    ac = _unT([r["o_acT"] for r in rs])
    dh = _unT([r["o_dhT"] for r in rs])
    dc = _unT([r["o_dcT"] for r in rs])
    aw = np.concatenate([r["o_aw"] for r in rs], 0)
    aw_cum = np.concatenate([r["o_awcum"] for r in rs], 0)
    ctx = np.concatenate([r["o_ctx"] for r in rs], 0)
    return (dec_out.astype(np.float32), gate.astype(np.float32),
            ah.astype(np.float32), ac.astype(np.float32),
            dh.astype(np.float32), dc.astype(np.float32),
            aw.astype(np.float32), aw_cum.astype(np.float32),
            ctx.astype(np.float32))


# revision 10
# speedup vs baseline: 1.0286x; 1.0286x over previous
"""Trainium2 Bass kernel for one Tacotron2-style decoder iteration.

Strategy: data-parallel over batch (B=256 -> 32 per core x 8 cores), all
weights replicated. Device math uses bf16 matmul operands with fp32 PSUM
accumulation; states/softmax stay fp32. Activations/weights are laid out
host-side in feature-major ("transposed") order so every engine op runs with
features on the SBUF partition dim.

Host-side work is limited to layout prep on constants/inputs: transposes,
dtype casts, concatenation (bias rows appended as extra matmul K rows),
conv->matmul folding of the location-attention weights, and im2col of the
attention-weight stack (pure gather).
"""

import numpy as np

import concourse.bass as bass
import concourse.mybir as mybir
from concourse import masks
from concourse.bass_utils import run_bass_kernel_spmd
from concourse.tile import TileContext, add_dep_helper

# ---------------------------------------------------------------- constants
B, T = 256, 512
MEL, PRE, ENC, ARNN, DRNN, ATT, LF, LK = 80, 256, 512, 1024, 1024, 128, 32, 31
PAD = (LK - 1) // 2
NCORES = 8
BL = B // NCORES  # 32 batch rows per core

K1 = 15   # LSTM1 contraction tiles: (256+512+1024+2 -> pad 1920)/128
K2 = 21   # LSTM2: (1024+512+1024+2 -> pad 2688)/128
KP = 13   # proj:  (1024+512+1 -> pad 1664)/128
NM = 32   # gate M-tiles (4096/128)
KH = ARNN // 128  # 8 hidden-state k-tiles
MG1 = 2   # W1 m-tiles per DMA group
MG2 = 2   # W2 m-tiles per DMA group
BG = 2    # memory batches per DMA group

# packed bf16 weight column offsets
_O_W1P = 0
_O_W2P = _O_W1P + 256
_O_QW = _O_W2P + 512
_O_M2 = _O_QW + KH * 128
_O_V = _O_M2 + 128
_O_PJ = _O_V + 1
NBF = _O_PJ + KP * 81

F32 = mybir.dt.float32
BF16 = mybir.dt.bfloat16


def _prenet_masks():
    """The reference regenerates its Bernoulli(0.5) prenet masks from
    jax.random.key(42) at call time; the realized bits depend on the jax
    backend, so compute them the same way (default backend) at run time."""
    import jax
    key = jax.random.key(42)
    return [
        np.asarray(
            jax.random.uniform(jax.random.fold_in(key, li), (PRE,)) <= 0.5
        ).astype(np.float32)
        for li in range(2)
    ]


def _bf(x):
    import ml_dtypes
    return np.ascontiguousarray(x).astype(ml_dtypes.bfloat16)


def _ktiles_pm(x, nk):
    """[nk*128, C] -> [128, nk, C] (partition-major k-tile layout)."""
    c = x.shape[1]
    return np.ascontiguousarray(x.reshape(nk, 128, c).transpose(1, 0, 2))


# ---------------------------------------------------------------- host prep
def _prep_shared(inp):
    """Layout prep shared across cores (weights). Returns dict of arrays."""
    import ml_dtypes
    pm0, pm1 = _prenet_masks()
    m0 = pm0 * 2.0
    m1 = pm1 * 2.0

    pk = np.zeros((128, NBF), ml_dtypes.bfloat16)
    pk[:80, _O_W1P:_O_W1P + 256] = _bf((inp["prenet_w1"] * m0[:, None]).T)
    pk[:, _O_W2P:_O_W2P + 512] = _ktiles_pm(
        _bf((inp["prenet_w2"] * m1[:, None]).T), 2).reshape(128, 512)
    pk[:, _O_QW:_O_QW + KH * 128] = _ktiles_pm(
        _bf(inp["q_w"].T), KH).reshape(128, KH * 128)
    M2 = inp["loc_lin_w"] @ inp["loc_conv_w"].reshape(LF, 2 * LK)  # [128, 62]
    pk[:2 * LK, _O_M2:_O_M2 + 128] = _bf(M2.T)
    pk[:ATT, _O_V:_O_V + 1] = _bf(inp["v_w"][0].reshape(ATT, 1))
    PJ = np.concatenate(
        [np.concatenate([inp["proj_w"], inp["gate_w"]], 0).T,
         np.concatenate([inp["proj_b"], inp["gate_b"]])[None],
         np.zeros((KP * 128 - 1537, 81), np.float32)], 0)      # [1664, 81]
    pk[:, _O_PJ:_O_PJ + KP * 81] = _ktiles_pm(_bf(PJ), KP).reshape(128, KP * 81)

    def lstm_tiles(w_ih, w_hh, b_ih, b_hh, nk, mg):
        W = np.concatenate(
            [w_ih.T, w_hh.T, b_ih[None], b_hh[None],
             np.zeros((nk * 128 - w_ih.shape[1] - w_hh.shape[1] - 2,
                       w_ih.shape[0]), np.float32)], 0)        # [nk*128, 4096]
        W = _bf(W)
        # [g][128 p][mg m][nk k][128 mcol]: one contiguous DMA per group
        out = np.empty((NM // mg, 128, mg, nk, 128), W.dtype)
        for m in range(NM):
            blk = W[:, m * 128:(m + 1) * 128]                  # [nk*128, 128]
            out[m // mg, :, m % mg] = blk.reshape(nk, 128, 128).transpose(1, 0, 2)
        return out

    W1g = lstm_tiles(inp["arnn_w_ih"], inp["arnn_w_hh"],
                     inp["arnn_b_ih"], inp["arnn_b_hh"], K1, MG1)
    W2g = lstm_tiles(inp["drnn_w_ih"], inp["drnn_w_hh"],
                     inp["drnn_b_ih"], inp["drnn_b_hh"], K2, MG2)
    return dict(pk=pk, W1g=W1g, W2g=W2g)


def _prep_percore(inp):
    """Per-core activation shards. Returns list of dicts (one per core)."""
    # im2col X[(c,k), b, t] = awc[b, c, t+k-15], zero padded
    awc = np.stack([inp["attention_weights"], inp["attention_weights_cum"]], 1)
    X = np.zeros((2 * LK, B, T), np.float32)
    for c in range(2):
        for k in range(LK):
            lo = max(0, PAD - k)
            hi = min(T, T + PAD - k)
            X[c * LK + k, :, lo:hi] = awc[:, c, lo + k - PAD: hi + k - PAD]
    Xb = _bf(X)

    pmT = _bf(inp["processed_memory"].transpose(2, 0, 1))      # [128, B, T]
    memb = _bf(inp["memory"])                                  # [B, T, ENC]

    # packed fp32 states: [128, 37, 32] chunks
    # 0: decoder_input^T (80 rows), 1-4: attention_context^T,
    # 5-12: ah_in, 13-20: ac_in, 21-28: dh_in, 29-36: dc_in
    pf = np.zeros((128, 37, B), np.float32)
    pf[:80, 0] = inp["decoder_input"].T
    pf[:, 1:5] = _ktiles_pm(np.ascontiguousarray(inp["attention_context"].T), 4)
    for i, nm in enumerate(("attention_hidden", "attention_cell",
                            "decoder_hidden", "decoder_cell")):
        pf[:, 5 + 8 * i:13 + 8 * i] = _ktiles_pm(
            np.ascontiguousarray(inp[nm].T), KH)

    shards = []
    for c in range(NCORES):
        s = slice(c * BL, (c + 1) * BL)
        m = np.ascontiguousarray(memb[s])                      # [32, 512, 512]
        mg = np.ascontiguousarray(
            m.reshape(BL // BG, BG, 4, 128, ENC).transpose(0, 3, 1, 2, 4))
        shards.append(dict(
            X=np.ascontiguousarray(Xb[:, s, :]).reshape(2 * LK, BL * T),
            pmT=np.ascontiguousarray(pmT[:, s, :]).reshape(128, BL * T),
            memg=mg,                                           # [16,128,8,512]
            pf=np.ascontiguousarray(pf[:, :, s]),
            awcum=np.ascontiguousarray(inp["attention_weights_cum"][s]),
        ))
    return shards


# ---------------------------------------------------------------- bass build
def build_kernel():
    nc = bass.Bass()
    p = {}
    def din(name, shape, dtype):
        p[name] = nc.declare_dram_parameter(name, list(shape), dtype, isOutput=False)
        return p[name]
    def dout(name, shape, dtype):
        p[name] = nc.declare_dram_parameter(name, list(shape), dtype, isOutput=True)
        return p[name]

    # inputs
    din("W1g", (NM // MG1, 128, MG1, K1, 128), BF16)
    din("W2g", (NM // MG2, 128, MG2, K2, 128), BF16)
    din("pk", (128, NBF), BF16)
    din("X", (2 * LK, BL * T), BF16)
    din("pmT", (128, BL * T), BF16)
    din("memg", (BL // BG, 128, BG * 4, ENC), BF16)
    din("pf", (128, 37, BL), F32)
    din("awcum", (BL, T), F32)
    # outputs
    dout("o_proj", (81, BL), F32)
    dout("o_ahT", (128, KH, BL), F32)
    dout("o_acT", (128, KH, BL), F32)
    dout("o_dhT", (128, KH, BL), F32)
    dout("o_dcT", (128, KH, BL), F32)
    dout("o_aw", (BL, T), F32)
    dout("o_awcum", (BL, T), F32)
    dout("o_ctx", (BL, ENC), F32)

    AF = mybir.ActivationFunctionType

    with TileContext(nc) as tc:
        with (
            tc.tile_pool(name="persist", bufs=1) as pp,
            tc.tile_pool(name="w1pool", bufs=3) as w1p_,
            tc.tile_pool(name="w2pool", bufs=2) as w2p_,
            tc.tile_pool(name="big", bufs=1) as bigp,
            tc.tile_pool(name="stream", bufs=2) as sp,
            tc.tile_pool(name="th", bufs=4) as thp,
            tc.tile_pool(name="psA", bufs=1, space="PSUM") as psA,
            tc.tile_pool(name="psB", bufs=2, space="PSUM") as psB,
        ):
            # ---- persistent small tiles / constants
            ident_bf = pp.tile([128, 128], BF16)
            masks.make_identity(nc, ident_bf[:])
            ident_f32 = pp.tile([32, 32], F32)
            masks.make_identity(nc, ident_f32[:])

            rhs1 = pp.tile([128, K1, BL], BF16)
            rhs2 = pp.tile([128, K2, BL], BF16)
            rhs_hc = pp.tile([128, KP, BL], BF16)
            nc.gpsimd.memset(rhs1[:, K1 - 1, :], 0.0)
            nc.gpsimd.memset(rhs2[:, K2 - 1, :], 0.0)
            nc.gpsimd.memset(rhs_hc[:, KP - 1, :], 0.0)
            nc.gpsimd.memset(rhs1[0:2, K1 - 1, :], 1.0)
            nc.gpsimd.memset(rhs2[0:2, K2 - 1, :], 1.0)
            nc.gpsimd.memset(rhs_hc[0:1, KP - 1, :], 1.0)

            # ---- packed small inputs
            pk_sb = pp.tile([128, NBF], BF16)
            nc.sync.dma_start(out=pk_sb[:], in_=p["pk"][:])
            pf_sb = pp.tile([128, 37, BL], F32)
            nc.sync.dma_start(out=pf_sb[:], in_=p["pf"][:])
            awcum_sb = pp.tile([BL, T], F32)
            nc.sync.dma_start(out=awcum_sb[:], in_=p["awcum"][:])

            w1p_sb = pk_sb[0:80, _O_W1P:_O_W1P + 256]
            w2p_v = pk_sb[:, _O_W2P:_O_W2P + 512].rearrange(
                "p (k c) -> p k c", k=2)
            qw_v = pk_sb[:, _O_QW:_O_QW + KH * 128].rearrange(
                "p (k c) -> p k c", k=KH)
            m2_sb = pk_sb[0:2 * LK, _O_M2:_O_M2 + 128]
            v_sb = pk_sb[0:ATT, _O_V:_O_V + 1]
            pj_v = pk_sb[:, _O_PJ:_O_PJ + KP * 81].rearrange(
                "p (k c) -> p k c", k=KP)

            # ---- prenet -> rhs1 chunks 0..1
            din_bf = pp.tile([80, BL], BF16)
            nc.vector.tensor_copy(din_bf[:], pf_sb[0:80, 0, :])
            ps_x = psB.tile([128, 2, BL], F32, tag="sm")
            for m in range(2):
                nc.tensor.matmul(ps_x[:, m, :], w1p_sb[:, m * 128:(m + 1) * 128],
                                 din_bf[:], start=True, stop=True)
            x1_bf = pp.tile([128, 2, BL], BF16)
            nc.scalar.activation(x1_bf[:], ps_x[:], AF.Relu)
            ps_x2 = psB.tile([128, 2, BL], F32, tag="sm")
            for m in range(2):
                for k in range(2):
                    nc.tensor.matmul(ps_x2[:, m, :],
                                     w2p_v[:, k, m * 128:(m + 1) * 128],
                                     x1_bf[:, k, :], start=(k == 0), stop=(k == 1))
            nc.scalar.activation(rhs1[:, 0:2, :], ps_x2[:], AF.Relu)

            # rhs1 chunks 2..5 = attention_context^T, 6..13 = ah_in^T (bf16)
            nc.vector.tensor_copy(rhs1[:, 2:6, :], pf_sb[:, 1:5, :])
            nc.vector.tensor_copy(rhs1[:, 6:6 + KH, :], pf_sb[:, 5:13, :])

            # ---- LSTM cell helper (returns last gate matmul instruction)
            def lstm(Wdram, nk, mg, wpool, rhs, c_in, o_cT, o_hT, h_bf_dst):
                ps_g = psA.tile([128, NM * BL], F32, tag="gates")
                last_mm = None
                for g in range(NM // mg):
                    w_sb = wpool.tile([128, mg, nk, 128], BF16, tag="wt")
                    nc.sync.dma_start(out=w_sb[:], in_=Wdram[g])
                    for mi in range(mg):
                        m = g * mg + mi
                        for k in range(nk):
                            last_mm = nc.tensor.matmul(
                                ps_g[:, m * BL:(m + 1) * BL],
                                w_sb[:, mi, k, :], rhs[:, k, :],
                                start=(k == 0), stop=(k == nk - 1))
                H = KH * BL  # 256 cols per gate group
                sig_if = pp.tile([128, 2 * H], F32, tag="sif")
                nc.scalar.activation(sig_if[:], ps_g[:, 0:2 * H], AF.Sigmoid)
                tan_g = pp.tile([128, H], F32, tag="tg")
                nc.scalar.activation(tan_g[:], ps_g[:, 2 * H:3 * H], AF.Tanh)
                sig_o = pp.tile([128, H], F32, tag="so")
                nc.scalar.activation(sig_o[:], ps_g[:, 3 * H:4 * H], AF.Sigmoid)
                c_new = pp.tile([128, KH, BL], F32, tag="cn")
                ig = pp.tile([128, H], F32, tag="ig")
                nc.vector.tensor_mul(ig[:], sig_if[:, 0:H], tan_g[:])
                cin_flat = c_in.rearrange("p k c -> p (k c)")
                fc = pp.tile([128, H], F32, tag="fc")
                nc.vector.tensor_mul(fc[:], sig_if[:, H:2 * H], cin_flat)
                cflat = c_new[:].rearrange("p k c -> p (k c)")
                nc.vector.tensor_add(cflat, ig[:], fc[:])
                nc.sync.dma_start(out=o_cT[:], in_=c_new[:])
                tc2 = pp.tile([128, H], F32, tag="tc2")
                nc.scalar.activation(tc2[:], cflat, AF.Tanh)
                h_new = pp.tile([128, KH, BL], F32, tag="hn")
                hflat = h_new[:].rearrange("p k c -> p (k c)")
                nc.vector.tensor_mul(hflat, sig_o[:], tc2[:])
                nc.sync.dma_start(out=o_hT[:], in_=h_new[:])
                nc.vector.tensor_copy(h_bf_dst, h_new[:])  # cast to bf16
                return last_mm

            l1_mm = lstm(p["W1g"], K1, MG1, w1p_, rhs1, pf_sb[:, 13:21, :],
                         p["o_acT"], p["o_ahT"], rhs2[:, 0:KH, :])

            # big attention operands: DMA-queued after W1, before mem/W2
            x_sb = bigp.tile([2 * LK, BL * T], BF16)
            nc.sync.dma_start(out=x_sb[:], in_=p["X"][:])
            pm_sb = bigp.tile([128, BL * T], BF16)
            nc.sync.dma_start(out=pm_sb[:], in_=p["pmT"][:])

            # ---- attention
            # pq = q_w @ ah  -> [128, BL] fp32 (per-partition bias for tanh)
            ps_pq = psB.tile([128, BL], F32, tag="sm")
            for k in range(KH):
                nc.tensor.matmul(ps_pq[:], qw_v[:, k, :], rhs2[:, k, :],
                                 start=(k == 0), stop=(k == KH - 1))
            pq_sb = pp.tile([128, BL], F32)
            nc.scalar.activation(pq_sb[:], ps_pq[:], AF.Copy)

            # v_diag[:, j, j] = v; energies rows land on partition b of one
            # shared [BL, T] PSUM bank via accumulation.
            v_diag = pp.tile([ATT, BL * BL], BF16)
            nc.gpsimd.memset(v_diag[:], 0.0)
            nc.vector.tensor_copy(v_diag[:][:, ::BL + 1],
                                  v_sb.broadcast_to((ATT, BL)))
            dv = v_diag[:].rearrange("p (b j) -> p b j", b=BL)

            ps_en = psA.tile([BL, T], F32, tag="enat")
            xv = x_sb[:].rearrange("p (b t) -> p b t", b=BL)
            pmv = pm_sb[:].rearrange("p (b t) -> p b t", b=BL)
            for b in range(BL):
                ps_arg = psB.tile([128, T], F32, tag="earg")
                mm = nc.tensor.matmul(ps_arg[:], m2_sb, xv[:, b, :],
                                      start=True, stop=False)
                if b == 0:
                    # keep the in-order PE queue from scheduling attention
                    # ahead of the LSTM1 gate matmuls
                    add_dep_helper(mm.ins, l1_mm.ins, sync=False,
                                   reason="PE order: attn after lstm1")
                nc.tensor.matmul(ps_arg[:], ident_bf[:], pmv[:, b, :],
                                 start=False, stop=True)
                th = thp.tile([128, T], BF16, tag="th")
                nc.scalar.activation(th[:], ps_arg[:], AF.Tanh,
                                     bias=pq_sb[:, b:b + 1])
                nc.tensor.matmul(ps_en[:], dv[:, b, :], th[:],
                                 start=(b == 0), stop=(b == BL - 1),
                                 skip_group_check=True)

            # softmax over t (free dim), rows = batch
            mx = pp.tile([BL, 1], F32)
            nc.vector.reduce_max(mx[:], ps_en[:], axis=mybir.AxisListType.X,
                                 negate=True)
            aw_sb = pp.tile([BL, T], F32)
            sm = pp.tile([BL, 1], F32)
            nc.scalar.activation(aw_sb[:], ps_en[:], AF.Exp, bias=mx[:],
                                 accum_out=sm[:])
            inv = pp.tile([BL, 1], F32)
            nc.vector.reciprocal(inv[:], sm[:])
            nc.vector.tensor_scalar_mul(aw_sb[:], aw_sb[:], inv[:])
            nc.sync.dma_start(out=p["o_aw"][:], in_=aw_sb[:])
            awcum_o = pp.tile([BL, T], F32)
            nc.vector.tensor_add(awcum_o[:], awcum_sb[:], aw_sb[:])
            nc.sync.dma_start(out=p["o_awcum"][:], in_=awcum_o[:])

            # aw^T [512, 32] as [128, 4, 32] bf16 via PE transpose
            awT_bf = pp.tile([128, 4, BL], BF16)
            for c in range(4):
                ps_t = psB.tile([128, BL], F32, tag="sm")
                nc.tensor.transpose(ps_t[:], aw_sb[:, c * 128:(c + 1) * 128],
                                    ident_f32[:])
                nc.vector.tensor_copy(awT_bf[:, c, :], ps_t[:])

            # aw^T diag blocks: awd[:, kt, b, j] = aw^T[kt, b] iff j==b;
            # ctx rows accumulate straight into a [BL, ENC] PSUM bank.
            awd = pp.tile([128, 4 * BL * BL], BF16)
            nc.gpsimd.memset(awd[:], 0.0)
            awd_diag = awd[:].rearrange("p (kt r) -> p kt r", kt=4)[:, :, ::BL + 1]
            nc.vector.tensor_copy(awd_diag, awT_bf[:])
            awdv = awd[:].rearrange("p (kt b j) -> p kt b j", kt=4, b=BL)

            ps_ctx = psA.tile([BL, ENC], F32, tag="ctx")
            ctx_mm = None
            for g in range(BL // BG):
                mem_g = sp.tile([128, BG * 4, ENC], BF16, tag="memb")
                nc.sync.dma_start(out=mem_g[:], in_=p["memg"][g])
                for bb in range(BG):
                    b = g * BG + bb
                    for kt in range(4):
                        ctx_mm = nc.tensor.matmul(
                            ps_ctx[:], awdv[:, kt, b, :],
                            mem_g[:, bb * 4 + kt, :],
                            start=(b == 0 and kt == 0),
                            stop=(b == BL - 1 and kt == 3),
                            skip_group_check=True)
            ctx_sb = pp.tile([BL, ENC], F32)
            nc.scalar.activation(ctx_sb[:], ps_ctx[:], AF.Copy)
            nc.sync.dma_start(out=p["o_ctx"][:], in_=ctx_sb[:])

            # ctx^T -> rhs2 chunks 8..11 (bf16)
            for c in range(4):
                ps_t = psB.tile([128, BL], F32, tag="sm")
                nc.tensor.transpose(ps_t[:], ctx_sb[:, c * 128:(c + 1) * 128],
                                    ident_f32[:])
                nc.vector.tensor_copy(rhs2[:, KH + c, :], ps_t[:])
            # rhs2 chunks 12..19 = dh_in^T
            nc.vector.tensor_copy(rhs2[:, KH + 4:KH + 4 + KH, :],
                                  pf_sb[:, 21:29, :])

            # ---- LSTM2 (PE-pinned after ctx)
            def lstm2_pin(mm):
                add_dep_helper(mm.ins, ctx_mm.ins, sync=False,
                               reason="PE order: lstm2 after ctx")
            l2_first = {}
            ps_g2 = psA.tile([128, NM * BL], F32, tag="gates")
            last2 = None
            for g in range(NM // MG2):
                w_sb = w2p_.tile([128, MG2, K2, 128], BF16, tag="wt2")
                nc.sync.dma_start(out=w_sb[:], in_=p["W2g"][g])
                for mi in range(MG2):
                    m = g * MG2 + mi
                    for k in range(K2):
                        last2 = nc.tensor.matmul(
                            ps_g2[:, m * BL:(m + 1) * BL],
                            w_sb[:, mi, k, :], rhs2[:, k, :],
                            start=(k == 0), stop=(k == K2 - 1))
                        if g == 0 and mi == 0 and k == 0:
                            lstm2_pin(last2)
            H = KH * BL
            sig_if = pp.tile([128, 2 * H], F32, tag="sif")
            nc.scalar.activation(sig_if[:], ps_g2[:, 0:2 * H], AF.Sigmoid)
            tan_g = pp.tile([128, H], F32, tag="tg")
            nc.scalar.activation(tan_g[:], ps_g2[:, 2 * H:3 * H], AF.Tanh)
            sig_o = pp.tile([128, H], F32, tag="so")
            nc.scalar.activation(sig_o[:], ps_g2[:, 3 * H:4 * H], AF.Sigmoid)
            c_new = pp.tile([128, KH, BL], F32, tag="cn")
            ig = pp.tile([128, H], F32, tag="ig")
            nc.vector.tensor_mul(ig[:], sig_if[:, 0:H], tan_g[:])
            fc = pp.tile([128, H], F32, tag="fc")
            nc.vector.tensor_mul(fc[:], sig_if[:, H:2 * H],
                                 pf_sb[:, 29:37, :].rearrange("p k c -> p (k c)"))
            cflat = c_new[:].rearrange("p k c -> p (k c)")
            nc.vector.tensor_add(cflat, ig[:], fc[:])
            nc.sync.dma_start(out=p["o_dcT"][:], in_=c_new[:])
            tc2_ = pp.tile([128, H], F32, tag="tc2")
            nc.scalar.activation(tc2_[:], cflat, AF.Tanh)
            h_new = pp.tile([128, KH, BL], F32, tag="hn")
            hflat = h_new[:].rearrange("p k c -> p (k c)")
            nc.vector.tensor_mul(hflat, sig_o[:], tc2_[:])
            nc.sync.dma_start(out=p["o_dhT"][:], in_=h_new[:])
            nc.vector.tensor_copy(rhs_hc[:, 0:KH, :], h_new[:])
            # rhs_hc chunks 8..11 = ctx^T
            nc.vector.tensor_copy(rhs_hc[:, KH:KH + 4, :], rhs2[:, KH:KH + 4, :])

            # ---- projection + gate
            ps_pr = psB.tile([81, BL], F32, tag="sm")
            for k in range(KP):
                nc.tensor.matmul(ps_pr[:], pj_v[:, k, 0:81], rhs_hc[:, k, :],
                                 start=(k == 0), stop=(k == KP - 1))
            pr_sb = pp.tile([81, BL], F32)
            nc.scalar.activation(pr_sb[:], ps_pr[:], AF.Copy)
            nc.sync.dma_start(out=p["o_proj"][:], in_=pr_sb[:])

    _fix_excess_waits(nc)
    return nc


def _fix_excess_waits(nc):
    """This walrus build rejects >1 sync-wait per instruction; spill extras
    into standalone InstEventSemaphore insts placed just before (same engine,
    so the stall semantics are identical)."""
    for f in nc.m.functions:
        for bb in f.blocks:
            insts = list(bb.instructions)
            out = []
            changed = False
            for ins in insts:
                si = ins.sync_info
                if (si is not None and si.on_wait and len(si.on_wait) > 1
                        and type(ins).__name__ != "InstEventSemaphore"):
                    waits = list(si.on_wait)
                    for j, w in enumerate(waits[1:]):
                        wi = mybir.InstEventSemaphore(
                            name=f"{ins.name}-xw{j}", ins=[], outs=[])
                        wi.engine = ins.engine
                        wi.sync_info = mybir.SyncInfo(on_wait=[w], on_update=[])
                        nc.register_instruction(wi, overwrite=True)
                        out.append(wi)
                        changed = True
                    ins.sync_info = mybir.SyncInfo(
                        on_wait=waits[:1], on_update=list(si.on_update or []))
                out.append(ins)
            if changed:
                bb.instructions = out


_CACHED = {}


def kernel(**inputs):
    inputs = {k: np.asarray(v) for k, v in inputs.items()}
    if "nc" not in _CACHED:
        _CACHED["nc"] = build_kernel()
    nc = _CACHED["nc"]

    shared = _prep_shared(inputs)
    shards = _prep_percore(inputs)
    in_maps = [dict(shared, **sh) for sh in shards]
    res = run_bass_kernel_spmd(nc, in_maps, list(range(NCORES)))

    def _unT(chunks):  # [128, KH, BL] per core -> [B, 1024]
        return np.concatenate(
            [r.transpose(1, 0, 2).reshape(ARNN, BL).T for r in chunks], 0)

    rs = res.results
    dec_out = np.concatenate([r["o_proj"][:80].T for r in rs], 0)
    gate = np.concatenate([r["o_proj"][80:81].T for r in rs], 0)
    ah = _unT([r["o_ahT"] for r in rs])
    ac = _unT([r["o_acT"] for r in rs])
    dh = _unT([r["o_dhT"] for r in rs])
    dc = _unT([r["o_dcT"] for r in rs])
    aw = np.concatenate([r["o_aw"] for r in rs], 0)
    aw_cum = np.concatenate([r["o_awcum"] for r in rs], 0)
    ctx = np.concatenate([r["o_ctx"] for r in rs], 0)
    return (dec_out.astype(np.float32), gate.astype(np.float32),
            ah.astype(np.float32), ac.astype(np.float32),
            dh.astype(np.float32), dc.astype(np.float32),
            aw.astype(np.float32), aw_cum.astype(np.float32),
            ctx.astype(np.float32))


# revision 11
# speedup vs baseline: 1.4717x; 1.4309x over previous
"""Trainium2 Bass kernel for one Tacotron2-style decoder iteration.

Strategy: data-parallel over batch (B=256 -> 32 per core x 8 cores), all
weights replicated. Device math uses bf16 matmul operands with fp32 PSUM
accumulation; states/softmax stay fp32. Activations/weights are laid out
host-side in feature-major ("transposed") order so every engine op runs with
features on the SBUF partition dim.

Host-side work is limited to layout prep on constants/inputs: transposes,
dtype casts, concatenation (bias rows appended as extra matmul K rows),
conv->matmul folding of the location-attention weights, and im2col of the
attention-weight stack (pure gather).
"""

import numpy as np

import concourse.bass as bass
import concourse.mybir as mybir
from concourse import masks
from concourse.bass_utils import run_bass_kernel_spmd
from concourse.tile import TileContext, add_dep_helper

# ---------------------------------------------------------------- constants
B, T = 256, 512
MEL, PRE, ENC, ARNN, DRNN, ATT, LF, LK = 80, 256, 512, 1024, 1024, 128, 32, 31
PAD = (LK - 1) // 2
NCORES = 8
BL = B // NCORES  # 32 batch rows per core

K1 = 15   # LSTM1 contraction tiles: (256+512+1024+2 -> pad 1920)/128
K2 = 21   # LSTM2: (1024+512+1024+2 -> pad 2688)/128
KP = 13   # proj:  (1024+512+1 -> pad 1664)/128
NM = 32   # gate M-tiles (4096/128)
KH = ARNN // 128  # 8 hidden-state k-tiles
MG1 = 2   # W1 m-tiles per DMA group
MG2 = 2   # W2 m-tiles per DMA group
BG = 2    # memory batches per DMA group

# packed bf16 weight column offsets
_O_W1P = 0
_O_W2P = _O_W1P + 256
_O_QW = _O_W2P + 512
_O_M2 = _O_QW + KH * 128
_O_V = _O_M2 + 128
_O_PJ = _O_V + 1
NBF = _O_PJ + KP * 81

F32 = mybir.dt.float32
BF16 = mybir.dt.bfloat16


def _prenet_masks():
    """The reference regenerates its Bernoulli(0.5) prenet masks from
    jax.random.key(42) at call time; the realized bits depend on the jax
    backend, so compute them the same way (default backend) at run time."""
    import jax
    key = jax.random.key(42)
    return [
        np.asarray(
            jax.random.uniform(jax.random.fold_in(key, li), (PRE,)) <= 0.5
        ).astype(np.float32)
        for li in range(2)
    ]


def _bf(x):
    import ml_dtypes
    return np.ascontiguousarray(x).astype(ml_dtypes.bfloat16)


def _ktiles_pm(x, nk):
    """[nk*128, C] -> [128, nk, C] (partition-major k-tile layout)."""
    c = x.shape[1]
    return np.ascontiguousarray(x.reshape(nk, 128, c).transpose(1, 0, 2))


# ---------------------------------------------------------------- host prep
def _prep_shared(inp):
    """Layout prep shared across cores (weights). Returns dict of arrays."""
    import ml_dtypes
    pm0, pm1 = _prenet_masks()
    m0 = pm0 * 2.0
    m1 = pm1 * 2.0

    pk = np.zeros((128, NBF), ml_dtypes.bfloat16)
    pk[:80, _O_W1P:_O_W1P + 256] = _bf((inp["prenet_w1"] * m0[:, None]).T)
    pk[:, _O_W2P:_O_W2P + 512] = _ktiles_pm(
        _bf((inp["prenet_w2"] * m1[:, None]).T), 2).reshape(128, 512)
    pk[:, _O_QW:_O_QW + KH * 128] = _ktiles_pm(
        _bf(inp["q_w"].T), KH).reshape(128, KH * 128)
    M2 = inp["loc_lin_w"] @ inp["loc_conv_w"].reshape(LF, 2 * LK)  # [128, 62]
    pk[:2 * LK, _O_M2:_O_M2 + 128] = _bf(M2.T)
    pk[:ATT, _O_V:_O_V + 1] = _bf(inp["v_w"][0].reshape(ATT, 1))
    PJ = np.concatenate(
        [np.concatenate([inp["proj_w"], inp["gate_w"]], 0).T,
         np.concatenate([inp["proj_b"], inp["gate_b"]])[None],
         np.zeros((KP * 128 - 1537, 81), np.float32)], 0)      # [1664, 81]
    pk[:, _O_PJ:_O_PJ + KP * 81] = _ktiles_pm(_bf(PJ), KP).reshape(128, KP * 81)

    def lstm_tiles(w_ih, w_hh, b_ih, b_hh, nk, mg):
        W = np.concatenate(
            [w_ih.T, w_hh.T, b_ih[None], b_hh[None],
             np.zeros((nk * 128 - w_ih.shape[1] - w_hh.shape[1] - 2,
                       w_ih.shape[0]), np.float32)], 0)        # [nk*128, 4096]
        W = _bf(W)
        # [g][128 p][mg m][nk k][128 mcol]: one contiguous DMA per group
        out = np.empty((NM // mg, 128, mg, nk, 128), W.dtype)
        for m in range(NM):
            blk = W[:, m * 128:(m + 1) * 128]                  # [nk*128, 128]
            out[m // mg, :, m % mg] = blk.reshape(nk, 128, 128).transpose(1, 0, 2)
        return out

    W1g = lstm_tiles(inp["arnn_w_ih"], inp["arnn_w_hh"],
                     inp["arnn_b_ih"], inp["arnn_b_hh"], K1, MG1)
    W2g = lstm_tiles(inp["drnn_w_ih"], inp["drnn_w_hh"],
                     inp["drnn_b_ih"], inp["drnn_b_hh"], K2, MG2)
    return dict(pk=pk, W1g=W1g, W2g=W2g)


def _prep_percore(inp):
    """Per-core activation shards. Returns list of dicts (one per core)."""
    # im2col X[(c,k), b, t] = awc[b, c, t+k-15], zero padded
    awc = np.stack([inp["attention_weights"], inp["attention_weights_cum"]], 1)
    X = np.zeros((2 * LK, B, T), np.float32)
    for c in range(2):
        for k in range(LK):
            lo = max(0, PAD - k)
            hi = min(T, T + PAD - k)
            X[c * LK + k, :, lo:hi] = awc[:, c, lo + k - PAD: hi + k - PAD]
    Xb = _bf(X)

    pmT = _bf(inp["processed_memory"].transpose(2, 0, 1))      # [128, B, T]
    memb = _bf(inp["memory"])                                  # [B, T, ENC]

    # packed fp32 states: [128, 37, 32] chunks
    # 0: decoder_input^T (80 rows), 1-4: attention_context^T,
    # 5-12: ah_in, 13-20: ac_in, 21-28: dh_in, 29-36: dc_in
    pf = np.zeros((128, 37, B), np.float32)
    pf[:80, 0] = inp["decoder_input"].T
    pf[:, 1:5] = _ktiles_pm(np.ascontiguousarray(inp["attention_context"].T), 4)
    for i, nm in enumerate(("attention_hidden", "attention_cell",
                            "decoder_hidden", "decoder_cell")):
        pf[:, 5 + 8 * i:13 + 8 * i] = _ktiles_pm(
            np.ascontiguousarray(inp[nm].T), KH)

    shards = []
    for c in range(NCORES):
        s = slice(c * BL, (c + 1) * BL)
        m = np.ascontiguousarray(memb[s])                      # [32, 512, 512]
        mg = np.ascontiguousarray(
            m.reshape(BL // BG, BG, 4, 128, ENC).transpose(0, 3, 1, 2, 4))
        shards.append(dict(
            X=np.ascontiguousarray(Xb[:, s, :]).reshape(2 * LK, BL * T),
            pmT=np.ascontiguousarray(pmT[:, s, :]).reshape(128, BL * T),
            memg=mg,                                           # [16,128,8,512]
            pf=np.ascontiguousarray(pf[:, :, s]),
            awcum=np.ascontiguousarray(inp["attention_weights_cum"][s]),
        ))
    return shards


# ---------------------------------------------------------------- bass build
def build_kernel():
    nc = bass.Bass()
    p = {}
    def din(name, shape, dtype):
        p[name] = nc.declare_dram_parameter(name, list(shape), dtype, isOutput=False)
        return p[name]
    def dout(name, shape, dtype):
        p[name] = nc.declare_dram_parameter(name, list(shape), dtype, isOutput=True)
        return p[name]

    # inputs
    din("W1g", (NM // MG1, 128, MG1, K1, 128), BF16)
    din("W2g", (NM // MG2, 128, MG2, K2, 128), BF16)
    din("pk", (128, NBF), BF16)
    din("X", (2 * LK, BL * T), BF16)
    din("pmT", (128, BL * T), BF16)
    din("memg", (BL // BG, 128, BG * 4, ENC), BF16)
    din("pf", (128, 37, BL), F32)
    din("awcum", (BL, T), F32)
    # outputs
    dout("o_proj", (81, BL), F32)
    dout("o_ahT", (128, KH, BL), F32)
    dout("o_acT", (128, KH, BL), F32)
    dout("o_dhT", (128, KH, BL), F32)
    dout("o_dcT", (128, KH, BL), F32)
    dout("o_aw", (BL, T), F32)
    dout("o_awcum", (BL, T), F32)
    dout("o_ctx", (BL, ENC), F32)

    AF = mybir.ActivationFunctionType

    with TileContext(nc) as tc:
        with (
            tc.tile_pool(name="persist", bufs=1) as pp,
            tc.tile_pool(name="w1pool", bufs=4) as w1p_,
            tc.tile_pool(name="w2pool", bufs=6) as w2p_,
            tc.tile_pool(name="big", bufs=2) as bigp,
            tc.tile_pool(name="stream", bufs=3) as sp,
            tc.tile_pool(name="th", bufs=4) as thp,
            tc.tile_pool(name="psA", bufs=1, space="PSUM") as psA,
            tc.tile_pool(name="psB", bufs=2, space="PSUM") as psB,
        ):
            # ---- persistent small tiles / constants
            ident_bf = pp.tile([128, 128], BF16)
            masks.make_identity(nc, ident_bf[:])
            ident_f32 = pp.tile([32, 32], F32)
            masks.make_identity(nc, ident_f32[:])

            rhs1 = pp.tile([128, K1, BL], BF16)
            rhs2 = pp.tile([128, K2, BL], BF16)
            rhs_hc = pp.tile([128, KP, BL], BF16)
            nc.gpsimd.memset(rhs1[:, K1 - 1, :], 0.0)
            nc.gpsimd.memset(rhs2[:, K2 - 1, :], 0.0)
            nc.gpsimd.memset(rhs_hc[:, KP - 1, :], 0.0)
            nc.gpsimd.memset(rhs1[0:2, K1 - 1, :], 1.0)
            nc.gpsimd.memset(rhs2[0:2, K2 - 1, :], 1.0)
            nc.gpsimd.memset(rhs_hc[0:1, KP - 1, :], 1.0)

            # ---- packed small inputs
            pk_sb = pp.tile([128, NBF], BF16)
            nc.sync.dma_start(out=pk_sb[:], in_=p["pk"][:])
            pf_sb = pp.tile([128, 37, BL], F32)
            nc.sync.dma_start(out=pf_sb[:], in_=p["pf"][:])
            awcum_sb = pp.tile([BL, T], F32)
            nc.sync.dma_start(out=awcum_sb[:], in_=p["awcum"][:])

            w1p_sb = pk_sb[0:80, _O_W1P:_O_W1P + 256]
            w2p_v = pk_sb[:, _O_W2P:_O_W2P + 512].rearrange(
                "p (k c) -> p k c", k=2)
            qw_v = pk_sb[:, _O_QW:_O_QW + KH * 128].rearrange(
                "p (k c) -> p k c", k=KH)
            m2_sb = pk_sb[0:2 * LK, _O_M2:_O_M2 + 128]
            v_sb = pk_sb[0:ATT, _O_V:_O_V + 1]
            pj_v = pk_sb[:, _O_PJ:_O_PJ + KP * 81].rearrange(
                "p (k c) -> p k c", k=KP)

            # ---- prenet -> rhs1 chunks 0..1
            din_bf = pp.tile([80, BL], BF16)
            nc.vector.tensor_copy(din_bf[:], pf_sb[0:80, 0, :])
            ps_x = psB.tile([128, 2, BL], F32, tag="sm")
            for m in range(2):
                nc.tensor.matmul(ps_x[:, m, :], w1p_sb[:, m * 128:(m + 1) * 128],
                                 din_bf[:], start=True, stop=True)
            x1_bf = pp.tile([128, 2, BL], BF16)
            nc.scalar.activation(x1_bf[:], ps_x[:], AF.Relu)
            ps_x2 = psB.tile([128, 2, BL], F32, tag="sm")
            for m in range(2):
                for k in range(2):
                    nc.tensor.matmul(ps_x2[:, m, :],
                                     w2p_v[:, k, m * 128:(m + 1) * 128],
                                     x1_bf[:, k, :], start=(k == 0), stop=(k == 1))
            nc.scalar.activation(rhs1[:, 0:2, :], ps_x2[:], AF.Relu)

            # rhs1 chunks 2..5 = attention_context^T, 6..13 = ah_in^T (bf16)
            nc.vector.tensor_copy(rhs1[:, 2:6, :], pf_sb[:, 1:5, :])
            nc.vector.tensor_copy(rhs1[:, 6:6 + KH, :], pf_sb[:, 5:13, :])

            # ---- LSTM cell helper (returns last gate matmul instruction)
            def lstm(Wdram, nk, mg, wpool, rhs, c_in, o_cT, o_hT, h_bf_dst):
                ps_g = psA.tile([128, NM * BL], F32, tag="gates")
                last_mm = None
                for g in range(NM // mg):
                    w_sb = wpool.tile([128, mg, nk, 128], BF16, tag="wt")
                    nc.sync.dma_start(out=w_sb[:], in_=Wdram[g])
                    for mi in range(mg):
                        m = g * mg + mi
                        for k in range(nk):
                            last_mm = nc.tensor.matmul(
                                ps_g[:, m * BL:(m + 1) * BL],
                                w_sb[:, mi, k, :], rhs[:, k, :],
                                start=(k == 0), stop=(k == nk - 1))
                H = KH * BL  # 256 cols per gate group
                sig_if = pp.tile([128, 2 * H], F32, tag="sif")
                nc.scalar.activation(sig_if[:], ps_g[:, 0:2 * H], AF.Sigmoid)
                tan_g = pp.tile([128, H], F32, tag="tg")
                nc.scalar.activation(tan_g[:], ps_g[:, 2 * H:3 * H], AF.Tanh)
                sig_o = pp.tile([128, H], F32, tag="so")
                nc.scalar.activation(sig_o[:], ps_g[:, 3 * H:4 * H], AF.Sigmoid)
                c_new = pp.tile([128, KH, BL], F32, tag="cn")
                ig = pp.tile([128, H], F32, tag="ig")
                nc.vector.tensor_mul(ig[:], sig_if[:, 0:H], tan_g[:])
                cin_flat = c_in.rearrange("p k c -> p (k c)")
                fc = pp.tile([128, H], F32, tag="fc")
                nc.vector.tensor_mul(fc[:], sig_if[:, H:2 * H], cin_flat)
                cflat = c_new[:].rearrange("p k c -> p (k c)")
                nc.vector.tensor_add(cflat, ig[:], fc[:])
                nc.sync.dma_start(out=o_cT[:], in_=c_new[:])
                tc2 = pp.tile([128, H], F32, tag="tc2")
                nc.scalar.activation(tc2[:], cflat, AF.Tanh)
                h_new = pp.tile([128, KH, BL], F32, tag="hn")
                hflat = h_new[:].rearrange("p k c -> p (k c)")
                nc.vector.tensor_mul(hflat, sig_o[:], tc2[:])
                nc.sync.dma_start(out=o_hT[:], in_=h_new[:])
                nc.vector.tensor_copy(h_bf_dst, h_new[:])  # cast to bf16
                return last_mm

            l1_mm = lstm(p["W1g"], K1, MG1, w1p_, rhs1, pf_sb[:, 13:21, :],
                         p["o_acT"], p["o_ahT"], rhs2[:, 0:KH, :])


            # ---- attention
            # pq = q_w @ ah  -> [128, BL] fp32 (per-partition bias for tanh)
            ps_pq = psB.tile([128, BL], F32, tag="sm")
            for k in range(KH):
                nc.tensor.matmul(ps_pq[:], qw_v[:, k, :], rhs2[:, k, :],
                                 start=(k == 0), stop=(k == KH - 1))
            pq_sb = pp.tile([128, BL], F32)
            nc.scalar.activation(pq_sb[:], ps_pq[:], AF.Copy)

            # v_diag[:, j, j] = v; energies rows land on partition b of one
            # shared [BL, T] PSUM bank via accumulation.
            v_diag = pp.tile([ATT, BL * BL], BF16)
            nc.gpsimd.memset(v_diag[:], 0.0)
            nc.vector.tensor_copy(v_diag[:][:, ::BL + 1],
                                  v_sb.broadcast_to((ATT, BL)))
            dv = v_diag[:].rearrange("p (b j) -> p b j", b=BL)

            ps_en = psA.tile([BL, T], F32, tag="enat")
            XB = 4  # batches per X/pm stream chunk
            for b in range(BL):
                if b % XB == 0:
                    x_ch = bigp.tile([2 * LK, XB * T], BF16, tag="xch")
                    nc.sync.dma_start(out=x_ch[:],
                                      in_=p["X"][:, b * T:(b + XB) * T])
                    pm_ch = bigp.tile([128, XB * T], BF16, tag="pmch")
                    nc.sync.dma_start(out=pm_ch[:],
                                      in_=p["pmT"][:, b * T:(b + XB) * T])
                j = b % XB
                ps_arg = psB.tile([128, T], F32, tag="earg")
                mm = nc.tensor.matmul(ps_arg[:], m2_sb,
                                      x_ch[:, j * T:(j + 1) * T],
                                      start=True, stop=False)
                if b == 0:
                    # keep the in-order PE queue from scheduling attention
                    # ahead of the LSTM1 gate matmuls
                    add_dep_helper(mm.ins, l1_mm.ins, sync=False,
                                   reason="PE order: attn after lstm1")
                nc.tensor.matmul(ps_arg[:], ident_bf[:],
                                 pm_ch[:, j * T:(j + 1) * T],
                                 start=False, stop=True)
                th = thp.tile([128, T], BF16, tag="th")
                nc.scalar.activation(th[:], ps_arg[:], AF.Tanh,
                                     bias=pq_sb[:, b:b + 1])
                nc.tensor.matmul(ps_en[:], dv[:, b, :], th[:],
                                 start=(b == 0), stop=(b == BL - 1),
                                 skip_group_check=True)

            # softmax over t (free dim), rows = batch
            mx = pp.tile([BL, 1], F32)
            nc.vector.reduce_max(mx[:], ps_en[:], axis=mybir.AxisListType.X,
                                 negate=True)
            aw_sb = pp.tile([BL, T], F32)
            sm = pp.tile([BL, 1], F32)
            nc.scalar.activation(aw_sb[:], ps_en[:], AF.Exp, bias=mx[:],
                                 accum_out=sm[:])
            inv = pp.tile([BL, 1], F32)
            nc.vector.reciprocal(inv[:], sm[:])
            nc.vector.tensor_scalar_mul(aw_sb[:], aw_sb[:], inv[:])
            nc.sync.dma_start(out=p["o_aw"][:], in_=aw_sb[:])
            awcum_o = pp.tile([BL, T], F32)
            nc.vector.tensor_add(awcum_o[:], awcum_sb[:], aw_sb[:])
            nc.sync.dma_start(out=p["o_awcum"][:], in_=awcum_o[:])

            # aw^T [512, 32] as [128, 4, 32] bf16 via PE transpose
            awT_bf = pp.tile([128, 4, BL], BF16)
            for c in range(4):
                ps_t = psB.tile([128, BL], F32, tag="sm")
                nc.tensor.transpose(ps_t[:], aw_sb[:, c * 128:(c + 1) * 128],
                                    ident_f32[:])
                nc.vector.tensor_copy(awT_bf[:, c, :], ps_t[:])

            # aw^T diag blocks: awd[:, kt, b, j] = aw^T[kt, b] iff j==b;
            # ctx rows accumulate straight into a [BL, ENC] PSUM bank.
            awd = pp.tile([128, 4 * BL * BL], BF16)
            nc.gpsimd.memset(awd[:], 0.0)
            awd_diag = awd[:].rearrange("p (kt r) -> p kt r", kt=4)[:, :, ::BL + 1]
            nc.vector.tensor_copy(awd_diag, awT_bf[:])
            awdv = awd[:].rearrange("p (kt b j) -> p kt b j", kt=4, b=BL)

            ps_ctx = psA.tile([BL, ENC], F32, tag="ctx")
            ctx_mm = None
            for g in range(BL // BG):
                mem_g = sp.tile([128, BG * 4, ENC], BF16, tag="memb")
                nc.sync.dma_start(out=mem_g[:], in_=p["memg"][g])
                for bb in range(BG):
                    b = g * BG + bb
                    for kt in range(4):
                        ctx_mm = nc.tensor.matmul(
                            ps_ctx[:], awdv[:, kt, b, :],
                            mem_g[:, bb * 4 + kt, :],
                            start=(b == 0 and kt == 0),
                            stop=(b == BL - 1 and kt == 3),
                            skip_group_check=True)
            ctx_sb = pp.tile([BL, ENC], F32)
            nc.scalar.activation(ctx_sb[:], ps_ctx[:], AF.Copy)
            nc.sync.dma_start(out=p["o_ctx"][:], in_=ctx_sb[:])

            # ctx^T -> rhs2 chunks 8..11 (bf16)
            for c in range(4):
                ps_t = psB.tile([128, BL], F32, tag="sm")
                nc.tensor.transpose(ps_t[:], ctx_sb[:, c * 128:(c + 1) * 128],
                                    ident_f32[:])
                nc.vector.tensor_copy(rhs2[:, KH + c, :], ps_t[:])
            # rhs2 chunks 12..19 = dh_in^T
            nc.vector.tensor_copy(rhs2[:, KH + 4:KH + 4 + KH, :],
                                  pf_sb[:, 21:29, :])

            # ---- LSTM2 (PE-pinned after ctx)
            def lstm2_pin(mm):
                add_dep_helper(mm.ins, ctx_mm.ins, sync=False,
                               reason="PE order: lstm2 after ctx")
            l2_first = {}
            ps_g2 = psA.tile([128, NM * BL], F32, tag="gates")
            last2 = None
            for g in range(NM // MG2):
                w_sb = w2p_.tile([128, MG2, K2, 128], BF16, tag="wt2")
                nc.sync.dma_start(out=w_sb[:], in_=p["W2g"][g])
                for mi in range(MG2):
                    m = g * MG2 + mi
                    for k in range(K2):
                        last2 = nc.tensor.matmul(
                            ps_g2[:, m * BL:(m + 1) * BL],
                            w_sb[:, mi, k, :], rhs2[:, k, :],
                            start=(k == 0), stop=(k == K2 - 1))
                        if g == 0 and mi == 0 and k == 0:
                            lstm2_pin(last2)
            H = KH * BL
            sig_if = pp.tile([128, 2 * H], F32, tag="sif")
            nc.scalar.activation(sig_if[:], ps_g2[:, 0:2 * H], AF.Sigmoid)
            tan_g = pp.tile([128, H], F32, tag="tg")
            nc.scalar.activation(tan_g[:], ps_g2[:, 2 * H:3 * H], AF.Tanh)
            sig_o = pp.tile([128, H], F32, tag="so")
            nc.scalar.activation(sig_o[:], ps_g2[:, 3 * H:4 * H], AF.Sigmoid)
            c_new = pp.tile([128, KH, BL], F32, tag="cn")
            ig = pp.tile([128, H], F32, tag="ig")
            nc.vector.tensor_mul(ig[:], sig_if[:, 0:H], tan_g[:])
            fc = pp.tile([128, H], F32, tag="fc")
            nc.vector.tensor_mul(fc[:], sig_if[:, H:2 * H],
                                 pf_sb[:, 29:37, :].rearrange("p k c -> p (k c)"))
            cflat = c_new[:].rearrange("p k c -> p (k c)")
            nc.vector.tensor_add(cflat, ig[:], fc[:])
            nc.sync.dma_start(out=p["o_dcT"][:], in_=c_new[:])
            tc2_ = pp.tile([128, H], F32, tag="tc2")
            nc.scalar.activation(tc2_[:], cflat, AF.Tanh)
            h_new = pp.tile([128, KH, BL], F32, tag="hn")
            hflat = h_new[:].rearrange("p k c -> p (k c)")
            nc.vector.tensor_mul(hflat, sig_o[:], tc2_[:])
            nc.sync.dma_start(out=p["o_dhT"][:], in_=h_new[:])
            nc.vector.tensor_copy(rhs_hc[:, 0:KH, :], h_new[:])
            # rhs_hc chunks 8..11 = ctx^T
            nc.vector.tensor_copy(rhs_hc[:, KH:KH + 4, :], rhs2[:, KH:KH + 4, :])

            # ---- projection + gate
            ps_pr = psB.tile([81, BL], F32, tag="sm")
            for k in range(KP):
                nc.tensor.matmul(ps_pr[:], pj_v[:, k, 0:81], rhs_hc[:, k, :],
                                 start=(k == 0), stop=(k == KP - 1))
            pr_sb = pp.tile([81, BL], F32)
            nc.scalar.activation(pr_sb[:], ps_pr[:], AF.Copy)
            nc.sync.dma_start(out=p["o_proj"][:], in_=pr_sb[:])

    _fix_excess_waits(nc)
    return nc


def _fix_excess_waits(nc):
    """This walrus build rejects >1 sync-wait per instruction; spill extras
    into standalone InstEventSemaphore insts placed just before (same engine,
    so the stall semantics are identical)."""
    for f in nc.m.functions:
        for bb in f.blocks:
            insts = list(bb.instructions)
            out = []
            changed = False
            for ins in insts:
                si = ins.sync_info
                if (si is not None and si.on_wait and len(si.on_wait) > 1
                        and type(ins).__name__ != "InstEventSemaphore"):
                    waits = list(si.on_wait)
                    for j, w in enumerate(waits[1:]):
                        wi = mybir.InstEventSemaphore(
                            name=f"{ins.name}-xw{j}", ins=[], outs=[])
                        wi.engine = ins.engine
                        wi.sync_info = mybir.SyncInfo(on_wait=[w], on_update=[])
                        nc.register_instruction(wi, overwrite=True)
                        out.append(wi)
                        changed = True
                    ins.sync_info = mybir.SyncInfo(
                        on_wait=waits[:1], on_update=list(si.on_update or []))
                out.append(ins)
            if changed:
                bb.instructions = out


_CACHED = {}


def kernel(**inputs):
    inputs = {k: np.asarray(v) for k, v in inputs.items()}
    if "nc" not in _CACHED:
        _CACHED["nc"] = build_kernel()
    nc = _CACHED["nc"]

    shared = _prep_shared(inputs)
    shards = _prep_percore(inputs)
    in_maps = [dict(shared, **sh) for sh in shards]
    res = run_bass_kernel_spmd(nc, in_maps, list(range(NCORES)))

    def _unT(chunks):  # [128, KH, BL] per core -> [B, 1024]
        return np.concatenate(
            [r.transpose(1, 0, 2).reshape(ARNN, BL).T for r in chunks], 0)

    rs = res.results
    dec_out = np.concatenate([r["o_proj"][:80].T for r in rs], 0)
    gate = np.concatenate([r["o_proj"][80:81].T for r in rs], 0)
    ah = _unT([r["o_ahT"] for r in rs])
    ac = _unT([r["o_acT"] for r in rs])
    dh = _unT([r["o_dhT"] for r in rs])
    dc = _unT([r["o_dcT"] for r in rs])
    aw = np.concatenate([r["o_aw"] for r in rs], 0)
    aw_cum = np.concatenate([r["o_awcum"] for r in rs], 0)
    ctx = np.concatenate([r["o_ctx"] for r in rs], 0)
    return (dec_out.astype(np.float32), gate.astype(np.float32),
            ah.astype(np.float32), ac.astype(np.float32),
            dh.astype(np.float32), dc.astype(np.float32),
            aw.astype(np.float32), aw_cum.astype(np.float32),
            ctx.astype(np.float32))
